# revision 6
# baseline (speedup 1.0000x reference)
"""Mixture-of-Experts (8 experts, top-2, D=1024, H=2048, T=8192) on 8 trn2 cores.

Strategy: expert-parallel with host-side routing.
  - Router (tiny: [T,D]@[D,E]) runs on host in float64; top-2 selection was
    verified to match fp32 jax (cpu + neuron) selection for this problem size.
  - Each core owns one expert and computes SwiGLU on only the tokens routed
    to it (~T*2/E rows instead of T), padded to capacity C.
  - Activations flow in transposed (feature-major) layout so the kernel needs
    no on-device transposes:
        h1T = w1 @ xT   (accumulate over D chunks)   [H, C]
        hT  = silu(h1T) * h3T                        [H, C]  (bf16)
        y   = (hT.T chunks) @ w2T                    [C, D]  (tokens on
              partitions so the per-token combine-weight scale is a
              per-partition tensor_scalar op)
  - Host combines: out[t] = y_e1[slot1] + y_e2[slot2] (cw applied on device).
"""

import sys
import types
from contextlib import ExitStack

import ml_dtypes
import numpy as np

import concourse.bass as bass
import concourse.tile as tile
from concourse import bacc, mybir
from concourse.bass_utils import run_bass_kernel_spmd


def install_axon_hooks_shim():
    """The container's antenv stub lacks axon_hooks, which
    run_bass_kernel_spmd imports whenever tracing is requested (including
    via the BASS_TRACE env var). Recreate it and register the NTFF
    profiling hook if the axon PJRT .so is present."""
    try:
        import antenv
    except ImportError:
        return False
    if "antenv.axon_hooks" in sys.modules:
        return sys.modules["antenv.axon_hooks"]._hook is not None
    mod = types.ModuleType("antenv.axon_hooks")
    mod._hook = None
    mod.set_axon_ntff_profile_hook = lambda h: setattr(mod, "_hook", h)
    mod.get_axon_ntff_profile_hook = lambda: mod._hook
    sys.modules["antenv.axon_hooks"] = mod
    antenv.axon_hooks = mod
    try:
        from trn_agent_boot.trn_boot import _ntff_profile_via_ctypes

        mod.set_axon_ntff_profile_hook(
            _ntff_profile_via_ctypes("/opt/axon/libaxon_pjrt.so")
        )
    except Exception:
        pass
    return mod._hook is not None


install_axon_hooks_shim()

E = 8  # experts == cores
D = 1024
H = 2048
TOP_K = 2

BF16 = mybir.dt.bfloat16
F16 = mybir.dt.float16
F32 = mybir.dt.float32

_CACHE: dict[int, object] = {}


def _route(x2d: np.ndarray, router_w: np.ndarray):
    """Float64 router. Returns per-expert token lists, per-expert combine
    weights, and for each token its (expert, slot-in-expert-batch) pairs."""
    T = x2d.shape[0]
    logits = x2d.astype(np.float64) @ router_w.astype(np.float64).T  # [T, E]
    order = np.argsort(-logits, axis=1, kind="stable")
    top2 = order[:, :TOP_K]  # [T, 2]
    lt = np.take_along_axis(logits, top2, axis=1)
    m = lt.max(axis=1, keepdims=True)
    ex = np.exp(lt - m)
    cw = (ex / ex.sum(axis=1, keepdims=True)).astype(np.float32)  # [T, 2]

    rows = []  # rows[e]: token ids routed to expert e (ascending)
    cw_e = []  # cw_e[e]: combine weight per routed token
    slot = np.empty((T, TOP_K), np.int64)  # slot[t, k]: row of t in expert batch
    for e in range(E):
        r = np.where((top2[:, 0] == e) | (top2[:, 1] == e))[0]
        k = np.where(top2[r, 0] == e, 0, 1)
        rows.append(r)
        cw_e.append(cw[r, k])
        slot[r, k] = np.arange(len(r))
    return rows, cw_e, top2, slot


# w1/w3 piece sizes in m-chunks (small first so early matmuls start early)
PIECES = (1, 1, 2, 4, 4, 4)


def _blocks_for(C):
    blocks = []
    t0 = 0
    while t0 < C:
        tb = min(512, C - t0)
        blocks.append((t0, tb))
        t0 += tb
    return blocks


def _build(C: int):
    """Build + compile the per-core Bass program for capacity C (mult of 128).

    All inputs are shipped pre-arranged in SBUF partition-major layout so
    every DMA is ~128 large contiguous descriptors (descriptor rate, not
    bandwidth, limits small strided transfers)."""
    assert C % 128 == 0
    nsub = C // 128  # token subtiles
    KA = D // 128  # 8 contraction chunks for matmul 1
    KM = H // 128  # 16 contraction chunks for matmul 2
    blocks = _blocks_for(C)
    NB = len(blocks)

    nc = bacc.Bacc("TRN2", target_bir_lowering=False, debug=False)

    # x, per token block, partition-major: [NB][128][KA][tb<=512]
    xtp = nc.declare_dram_parameter("xtp", [NB, 128, KA, 512], BF16, isOutput=False)
    w1ps = [
        nc.declare_dram_parameter(f"w1p{p}", [128, KA, sz * 128], BF16, isOutput=False)
        for p, sz in enumerate(PIECES)
    ]
    w3ps = [
        nc.declare_dram_parameter(f"w3p{p}", [128, KA, sz * 128], BF16, isOutput=False)
        for p, sz in enumerate(PIECES)
    ]
    w2ps = [
        [
            nc.declare_dram_parameter(
                f"w2p{mh}{dh}", [128, KM // 2, 512], BF16, isOutput=False
            )
            for dh in range(2)
        ]
        for mh in range(2)
    ]
    cwt = nc.declare_dram_parameter("cwt", [128, nsub], F32, isOutput=False)
    # f16 output: halves the output DMA (and its tail after the last matmul);
    # ~0.05% relative rounding on y, well within budget.
    y = nc.declare_dram_parameter("y", [C, D], F16, isOutput=True)
    y_v = y.rearrange("(n p) d -> n p d", p=128)  # [nsub, 128, D]

    with ExitStack() as ctx:
        tc = ctx.enter_context(tile.TileContext(nc))
        wpool = ctx.enter_context(tc.tile_pool(name="weights", bufs=1))
        xpool = ctx.enter_context(tc.tile_pool(name="x", bufs=3))
        hpool = ctx.enter_context(tc.tile_pool(name="h", bufs=2))
        spool = ctx.enter_context(tc.tile_pool(name="s", bufs=3))
        ypool = ctx.enter_context(tc.tile_pool(name="y", bufs=4))
        ppool = ctx.enter_context(tc.tile_pool(name="psum", bufs=2, space="PSUM"))

        # first token block's activations first — they gate the first matmul.
        # The first block (and first weight piece) are split into per-KA-chunk
        # DMAs so the very first matmul only waits on a 128KB transfer instead
        # of the whole 1MB tile (startup latency: ~12µs -> ~3µs).
        def xts_load(bi, tb, split=False):
            xa = xpool.tile([128, KA, tb], BF16, tag="xts")
            if split:
                for a in range(KA):
                    nc.sync.dma_start(xa[:, a, :], xtp[bi, :, a, 0:tb])
            else:
                nc.sync.dma_start(xa[:], xtp[bi, :, :, 0:tb])
            return xa

        xts0 = xts_load(0, blocks[0][1], split=True)

        w1p, w3p = [], []  # per m-chunk: (tile, offset)
        for p, sz in enumerate(PIECES):
            t1 = wpool.tile([128, KA, sz * 128], BF16, tag=f"w1s{p}")
            t3 = wpool.tile([128, KA, sz * 128], BF16, tag=f"w3s{p}")
            if p == 0:
                for a in range(KA):
                    nc.sync.dma_start(t1[:, a, :], w1ps[p][:, a, :])
                for a in range(KA):
                    nc.sync.dma_start(t3[:, a, :], w3ps[p][:, a, :])
            else:
                nc.sync.dma_start(t1[:], w1ps[p][:])
                nc.sync.dma_start(t3[:], w3ps[p][:])
            for i in range(sz):
                w1p.append((t1, i))
                w3p.append((t3, i))

        w2p = []  # [m-half][d-half] tiles of [128, KM//2, 512]
        for mh in range(2):
            row = []
            for dh in range(2):
                t2 = wpool.tile([128, KM // 2, 512], BF16, tag=f"w2s{mh}{dh}")
                nc.sync.dma_start(t2[:], w2ps[mh][dh][:])
                row.append(t2)
            w2p.append(row)
        cws = wpool.tile([128, nsub], F32, tag="cws")
        nc.sync.dma_start(cws[:], cwt[:])

        for bi, (t0, tb) in enumerate(blocks):
            xts = xts0 if bi == 0 else xts_load(bi, tb)
            xsl = lambda a, xts=xts: xts[:, a, :]

            hts = hpool.tile([128, KM, tb], BF16, tag="hts")

            # phase A: h1T/h3T chunks + silu * mul -> hts
            for m in range(KM):
                w1s, o1 = w1p[m]
                w3s, o3 = w3p[m]
                ph1 = ppool.tile([128, tb], F32, tag="ph1")
                for a in range(KA):
                    nc.tensor.matmul(
                        ph1[:],
                        w1s[:, a, bass.ts(o1, 128)],
                        xsl(a),
                        start=(a == 0),
                        stop=(a == KA - 1),
                    )
                ph3 = ppool.tile([128, tb], F32, tag="ph3")
                for a in range(KA):
                    nc.tensor.matmul(
                        ph3[:],
                        w3s[:, a, bass.ts(o3, 128)],
                        xsl(a),
                        start=(a == 0),
                        stop=(a == KA - 1),
                    )
                sil = spool.tile([128, tb], BF16, tag="sil")
                nc.scalar.activation(
                    sil[:], ph1[:], mybir.ActivationFunctionType.Silu
                )
                nc.vector.tensor_mul(hts[:, m, :], sil[:], ph3[:])

            # phase B: y = hT.T @ w2T, scaled by cw
            for n in range(tb // 128):
                nsl = bass.ts(n, 128)
                gn = t0 // 128 + n  # global subtile index
                py0 = ppool.tile([128, 512], F32, tag="py0")
                py1 = ppool.tile([128, 512], F32, tag="py1")
                for m in range(KM):
                    mh, mr = divmod(m, KM // 2)
                    nc.tensor.matmul(
                        py0[:],
                        hts[:, m, nsl],
                        w2p[mh][0][:, mr, :],
                        start=(m == 0),
                        stop=(m == KM - 1),
                    )
                    nc.tensor.matmul(
                        py1[:],
                        hts[:, m, nsl],
                        w2p[mh][1][:, mr, :],
                        start=(m == 0),
                        stop=(m == KM - 1),
                    )
                ys0 = ypool.tile([128, 512], F16, tag="ys0")
                nc.vector.tensor_scalar_mul(ys0[:], py0[:], cws[:, gn : gn + 1])
                nc.sync.dma_start(y_v[gn][:, 0:512], ys0[:])
                ys1 = ypool.tile([128, 512], F16, tag="ys1")
                nc.vector.tensor_scalar_mul(ys1[:], py1[:], cws[:, gn : gn + 1])
                nc.sync.dma_start(y_v[gn][:, 512:1024], ys1[:])

    nc.compile()
    return nc


def _get(C: int):
    if C not in _CACHE:
        _CACHE[C] = _build(C)
    return _CACHE[C]


def _prepare_core_inputs(x2d, w1, w2, w3, rows, cw_e, C):
    bf = ml_dtypes.bfloat16
    nsub = C // 128
    KA, KM = D // 128, H // 128
    blocks = _blocks_for(C)
    NB = len(blocks)
    in_maps = []
    for e in range(E):
        ce = len(rows[e])
        xt = np.zeros((D, C), bf)
        xt[:, :ce] = x2d[rows[e]].T.astype(bf)
        # partition-major per-block: [NB, 128, KA, 512]
        xpm = xt.reshape(KA, 128, C).transpose(1, 0, 2)  # [128, KA, C]
        xtp = np.zeros((NB, 128, KA, 512), bf)
        for b, (t0, tb) in enumerate(blocks):
            xtp[b, :, :, :tb] = xpm[:, :, t0 : t0 + tb]

        w1pm = w1[e].T.astype(bf).reshape(KA, 128, H).transpose(1, 0, 2)
        w3pm = w3[e].T.astype(bf).reshape(KA, 128, H).transpose(1, 0, 2)
        w2pm = w2[e].T.astype(bf).reshape(KM, 128, D).transpose(1, 0, 2)

        cwt = np.zeros((C,), np.float32)
        cwt[:ce] = cw_e[e]

        m = {"xtp": xtp, "cwt": np.ascontiguousarray(cwt.reshape(nsub, 128).T)}
        m0 = 0
        for p, sz in enumerate(PIECES):
            hs = slice(m0 * 128, (m0 + sz) * 128)
            m[f"w1p{p}"] = np.ascontiguousarray(w1pm[:, :, hs])
            m[f"w3p{p}"] = np.ascontiguousarray(w3pm[:, :, hs])
            m0 += sz
        for mh in range(2):
            msl = slice(mh * (KM // 2), (mh + 1) * (KM // 2))
            for dh in range(2):
                m[f"w2p{mh}{dh}"] = np.ascontiguousarray(
                    w2pm[:, msl, dh * 512 : (dh + 1) * 512]
                )
        in_maps.append(m)
    return in_maps


def run(inputs: dict, trace: bool = False, trace_cores=None):
    """Core implementation; returns (output, BassKernelResults)."""
    x = np.asarray(inputs["x"])
    router_w = np.asarray(inputs["router_w"], np.float32)
    w1 = np.asarray(inputs["w1"], np.float32)
    w2 = np.asarray(inputs["w2"], np.float32)
    w3 = np.asarray(inputs["w3"], np.float32)

    B, S, _ = x.shape
    assert x.shape[-1] == D and router_w.shape == (E, D), (x.shape, router_w.shape)
    assert w1.shape == (E, H, D) and w3.shape == (E, H, D) and w2.shape == (E, D, H)
    x2d = np.ascontiguousarray(x.reshape(-1, D).astype(np.float32))
    T = x2d.shape[0]

    rows, cw_e, top2, slot = _route(x2d, router_w)
    cmax = max(len(r) for r in rows)
    C = max(128, int(np.ceil(cmax / 128) * 128))

    nc = _get(C)
    in_maps = _prepare_core_inputs(x2d, w1, w2, w3, rows, cw_e, C)
    res = run_bass_kernel_spmd(
        nc,
        in_maps,
        list(range(E)),
        trace=trace,
        trace_cores=trace_cores,
    )

    Y = np.stack([np.asarray(res.results[e]["y"], np.float32) for e in range(E)])
    Yf = Y.reshape(E * C, D)  # [E*C, D] f32 (device emits f16)
    fi = top2.astype(np.int64) * C + slot  # [T, 2]
    out = Yf[fi[:, 0]] + Yf[fi[:, 1]]
    return out.reshape(B, S, D).astype(x.dtype), res


def kernel(**inputs) -> np.ndarray:
    out, _ = run(inputs, trace=False)
    return out



# revision 7
# speedup vs baseline: 1.0660x; 1.0660x over previous
"""Mixture-of-Experts (8 experts, top-2, D=1024, H=2048, T=8192) on 8 trn2 cores.

Strategy: expert-parallel with host-side routing.
  - Router (tiny: [T,D]@[D,E]) runs on host in float64; top-2 selection was
    verified to match fp32 jax (cpu + neuron) selection for this problem size.
  - Each core owns one expert and computes SwiGLU on only the tokens routed
    to it (~T*2/E rows instead of T), padded to capacity C.
  - Activations flow in transposed (feature-major) layout so the kernel needs
    no on-device transposes:
        h1T = w1 @ xT   (accumulate over D chunks)   [H, C]
        hT  = silu(h1T) * h3T                        [H, C]  (bf16)
        y   = (hT.T chunks) @ w2T                    [C, D]  (tokens on
              partitions so the per-token combine-weight scale is a
              per-partition tensor_scalar op)
  - Host combines: out[t] = y_e1[slot1] + y_e2[slot2] (cw applied on device).
"""

import sys
import types
from contextlib import ExitStack

import ml_dtypes
import numpy as np

import concourse.bass as bass
import concourse.tile as tile
from concourse import bacc, mybir
from concourse.bass_utils import run_bass_kernel_spmd


def install_axon_hooks_shim():
    """The container's antenv stub lacks axon_hooks, which
    run_bass_kernel_spmd imports whenever tracing is requested (including
    via the BASS_TRACE env var). Recreate it and register the NTFF
    profiling hook if the axon PJRT .so is present."""
    try:
        import antenv
    except ImportError:
        return False
    if "antenv.axon_hooks" in sys.modules:
        return sys.modules["antenv.axon_hooks"]._hook is not None
    mod = types.ModuleType("antenv.axon_hooks")
    mod._hook = None
    mod.set_axon_ntff_profile_hook = lambda h: setattr(mod, "_hook", h)
    mod.get_axon_ntff_profile_hook = lambda: mod._hook
    sys.modules["antenv.axon_hooks"] = mod
    antenv.axon_hooks = mod
    try:
        from trn_agent_boot.trn_boot import _ntff_profile_via_ctypes

        mod.set_axon_ntff_profile_hook(
            _ntff_profile_via_ctypes("/opt/axon/libaxon_pjrt.so")
        )
    except Exception:
        pass
    return mod._hook is not None


install_axon_hooks_shim()

E = 8  # experts == cores
D = 1024
H = 2048
TOP_K = 2

BF16 = mybir.dt.bfloat16
F16 = mybir.dt.float16
F32 = mybir.dt.float32

_CACHE: dict[int, object] = {}


def _route(x2d: np.ndarray, router_w: np.ndarray):
    """Float64 router. Returns per-expert token lists, per-expert combine
    weights, and for each token its (expert, slot-in-expert-batch) pairs."""
    T = x2d.shape[0]
    logits = x2d.astype(np.float64) @ router_w.astype(np.float64).T  # [T, E]
    order = np.argsort(-logits, axis=1, kind="stable")
    top2 = order[:, :TOP_K]  # [T, 2]
    lt = np.take_along_axis(logits, top2, axis=1)
    m = lt.max(axis=1, keepdims=True)
    ex = np.exp(lt - m)
    cw = (ex / ex.sum(axis=1, keepdims=True)).astype(np.float32)  # [T, 2]

    rows = []  # rows[e]: token ids routed to expert e (ascending)
    cw_e = []  # cw_e[e]: combine weight per routed token
    slot = np.empty((T, TOP_K), np.int64)  # slot[t, k]: row of t in expert batch
    for e in range(E):
        r = np.where((top2[:, 0] == e) | (top2[:, 1] == e))[0]
        k = np.where(top2[r, 0] == e, 0, 1)
        rows.append(r)
        cw_e.append(cw[r, k])
        slot[r, k] = np.arange(len(r))
    return rows, cw_e, top2, slot


# w1/w3 piece sizes in m-chunks (small first so early matmuls start early)
PIECES = (1, 1, 2, 4, 4, 4)


def _blocks_for(C):
    blocks = []
    t0 = 0
    while t0 < C:
        tb = min(512, C - t0)
        blocks.append((t0, tb))
        t0 += tb
    return blocks


def _build(C: int):
    """Build + compile the per-core Bass program for capacity C (mult of 128).

    All inputs are shipped pre-arranged in SBUF partition-major layout so
    every DMA is ~128 large contiguous descriptors (descriptor rate, not
    bandwidth, limits small strided transfers)."""
    assert C % 128 == 0
    nsub = C // 128  # token subtiles
    KA = D // 128  # 8 contraction chunks for matmul 1
    KM = H // 128  # 16 contraction chunks for matmul 2
    blocks = _blocks_for(C)
    NB = len(blocks)

    nc = bacc.Bacc("TRN2", target_bir_lowering=False, debug=False)

    # x, per token block, partition-major: [NB][128][KA][tb<=512]
    xtp = nc.declare_dram_parameter("xtp", [NB, 128, KA, 512], BF16, isOutput=False)
    w1ps = [
        nc.declare_dram_parameter(f"w1p{p}", [128, KA, sz * 128], BF16, isOutput=False)
        for p, sz in enumerate(PIECES)
    ]
    w3ps = [
        nc.declare_dram_parameter(f"w3p{p}", [128, KA, sz * 128], BF16, isOutput=False)
        for p, sz in enumerate(PIECES)
    ]
    w2ps = [
        [
            nc.declare_dram_parameter(
                f"w2p{mh}{dh}", [128, KM // 2, 512], BF16, isOutput=False
            )
            for dh in range(2)
        ]
        for mh in range(2)
    ]
    cwt = nc.declare_dram_parameter("cwt", [128, nsub], F32, isOutput=False)
    # f16 output: halves the output DMA (and its tail after the last matmul);
    # ~0.05% relative rounding on y, well within budget.
    y = nc.declare_dram_parameter("y", [C, D], F16, isOutput=True)
    y_v = y.rearrange("(n p) d -> n p d", p=128)  # [nsub, 128, D]

    with ExitStack() as ctx:
        tc = ctx.enter_context(tile.TileContext(nc))
        wpool = ctx.enter_context(tc.tile_pool(name="weights", bufs=1))
        xpool = ctx.enter_context(tc.tile_pool(name="x", bufs=3))
        hpool = ctx.enter_context(tc.tile_pool(name="h", bufs=2))
        spool = ctx.enter_context(tc.tile_pool(name="s", bufs=3))
        ypool = ctx.enter_context(tc.tile_pool(name="y", bufs=4))
        ppool = ctx.enter_context(tc.tile_pool(name="psum", bufs=2, space="PSUM"))

        # first token block's activations first — they gate the first matmul
        def xts_load(bi, tb):
            xa = xpool.tile([128, KA, tb], BF16, tag="xts")
            nc.sync.dma_start(xa[:], xtp[bi, :, :, 0:tb])
            return xa

        xts0 = xts_load(0, blocks[0][1])

        w1p, w3p = [], []  # per m-chunk: (tile, offset)
        for p, sz in enumerate(PIECES):
            t1 = wpool.tile([128, KA, sz * 128], BF16, tag=f"w1s{p}")
            nc.sync.dma_start(t1[:], w1ps[p][:])
            t3 = wpool.tile([128, KA, sz * 128], BF16, tag=f"w3s{p}")
            nc.sync.dma_start(t3[:], w3ps[p][:])
            for i in range(sz):
                w1p.append((t1, i))
                w3p.append((t3, i))

        w2p = []  # [m-half][d-half] tiles of [128, KM//2, 512]
        for mh in range(2):
            row = []
            for dh in range(2):
                t2 = wpool.tile([128, KM // 2, 512], BF16, tag=f"w2s{mh}{dh}")
                nc.sync.dma_start(t2[:], w2ps[mh][dh][:])
                row.append(t2)
            w2p.append(row)
        cws = wpool.tile([128, nsub], F32, tag="cws")
        nc.sync.dma_start(cws[:], cwt[:])

        for bi, (t0, tb) in enumerate(blocks):
            xts = xts0 if bi == 0 else xts_load(bi, tb)
            xsl = lambda a, xts=xts: xts[:, a, :]

            hts = hpool.tile([128, KM, tb], BF16, tag="hts")

            # phase A: h1T/h3T chunks + silu * mul -> hts
            for m in range(KM):
                w1s, o1 = w1p[m]
                w3s, o3 = w3p[m]
                ph1 = ppool.tile([128, tb], F32, tag="ph1")
                for a in range(KA):
                    nc.tensor.matmul(
                        ph1[:],
                        w1s[:, a, bass.ts(o1, 128)],
                        xsl(a),
                        start=(a == 0),
                        stop=(a == KA - 1),
                    )
                ph3 = ppool.tile([128, tb], F32, tag="ph3")
                for a in range(KA):
                    nc.tensor.matmul(
                        ph3[:],
                        w3s[:, a, bass.ts(o3, 128)],
                        xsl(a),
                        start=(a == 0),
                        stop=(a == KA - 1),
                    )
                sil = spool.tile([128, tb], BF16, tag="sil")
                nc.scalar.activation(
                    sil[:], ph1[:], mybir.ActivationFunctionType.Silu
                )
                nc.vector.tensor_mul(hts[:, m, :], sil[:], ph3[:])

            # phase B: y = hT.T @ w2T, scaled by cw
            for n in range(tb // 128):
                nsl = bass.ts(n, 128)
                gn = t0 // 128 + n  # global subtile index
                py0 = ppool.tile([128, 512], F32, tag="py0")
                py1 = ppool.tile([128, 512], F32, tag="py1")
                for m in range(KM):
                    mh, mr = divmod(m, KM // 2)
                    nc.tensor.matmul(
                        py0[:],
                        hts[:, m, nsl],
                        w2p[mh][0][:, mr, :],
                        start=(m == 0),
                        stop=(m == KM - 1),
                    )
                    nc.tensor.matmul(
                        py1[:],
                        hts[:, m, nsl],
                        w2p[mh][1][:, mr, :],
                        start=(m == 0),
                        stop=(m == KM - 1),
                    )
                ys0 = ypool.tile([128, 512], F16, tag="ys0")
                nc.vector.tensor_scalar_mul(ys0[:], py0[:], cws[:, gn : gn + 1])
                nc.sync.dma_start(y_v[gn][:, 0:512], ys0[:])
                ys1 = ypool.tile([128, 512], F16, tag="ys1")
                nc.vector.tensor_scalar_mul(ys1[:], py1[:], cws[:, gn : gn + 1])
                nc.sync.dma_start(y_v[gn][:, 512:1024], ys1[:])

    nc.compile()
    return nc


def _get(C: int):
    if C not in _CACHE:
        _CACHE[C] = _build(C)
    return _CACHE[C]


def _prepare_core_inputs(x2d, w1, w2, w3, rows, cw_e, C):
    bf = ml_dtypes.bfloat16
    nsub = C // 128
    KA, KM = D // 128, H // 128
    blocks = _blocks_for(C)
    NB = len(blocks)
    in_maps = []
    for e in range(E):
        ce = len(rows[e])
        xt = np.zeros((D, C), bf)
        xt[:, :ce] = x2d[rows[e]].T.astype(bf)
        # partition-major per-block: [NB, 128, KA, 512]
        xpm = xt.reshape(KA, 128, C).transpose(1, 0, 2)  # [128, KA, C]
        xtp = np.zeros((NB, 128, KA, 512), bf)
        for b, (t0, tb) in enumerate(blocks):
            xtp[b, :, :, :tb] = xpm[:, :, t0 : t0 + tb]

        w1pm = w1[e].T.astype(bf).reshape(KA, 128, H).transpose(1, 0, 2)
        w3pm = w3[e].T.astype(bf).reshape(KA, 128, H).transpose(1, 0, 2)
        w2pm = w2[e].T.astype(bf).reshape(KM, 128, D).transpose(1, 0, 2)

        cwt = np.zeros((C,), np.float32)
        cwt[:ce] = cw_e[e]

        m = {"xtp": xtp, "cwt": np.ascontiguousarray(cwt.reshape(nsub, 128).T)}
        m0 = 0
        for p, sz in enumerate(PIECES):
            hs = slice(m0 * 128, (m0 + sz) * 128)
            m[f"w1p{p}"] = np.ascontiguousarray(w1pm[:, :, hs])
            m[f"w3p{p}"] = np.ascontiguousarray(w3pm[:, :, hs])
            m0 += sz
        for mh in range(2):
            msl = slice(mh * (KM // 2), (mh + 1) * (KM // 2))
            for dh in range(2):
                m[f"w2p{mh}{dh}"] = np.ascontiguousarray(
                    w2pm[:, msl, dh * 512 : (dh + 1) * 512]
                )
        in_maps.append(m)
    return in_maps


def run(inputs: dict, trace: bool = False, trace_cores=None):
    """Core implementation; returns (output, BassKernelResults)."""
    x = np.asarray(inputs["x"])
    router_w = np.asarray(inputs["router_w"], np.float32)
    w1 = np.asarray(inputs["w1"], np.float32)
    w2 = np.asarray(inputs["w2"], np.float32)
    w3 = np.asarray(inputs["w3"], np.float32)

    B, S, _ = x.shape
    assert x.shape[-1] == D and router_w.shape == (E, D), (x.shape, router_w.shape)
    assert w1.shape == (E, H, D) and w3.shape == (E, H, D) and w2.shape == (E, D, H)
    x2d = np.ascontiguousarray(x.reshape(-1, D).astype(np.float32))
    T = x2d.shape[0]

    rows, cw_e, top2, slot = _route(x2d, router_w)
    cmax = max(len(r) for r in rows)
    C = max(128, int(np.ceil(cmax / 128) * 128))

    nc = _get(C)
    in_maps = _prepare_core_inputs(x2d, w1, w2, w3, rows, cw_e, C)
    res = run_bass_kernel_spmd(
        nc,
        in_maps,
        list(range(E)),
        trace=trace,
        trace_cores=trace_cores,
    )

    Y = np.stack([np.asarray(res.results[e]["y"], np.float32) for e in range(E)])
    Yf = Y.reshape(E * C, D)  # [E*C, D] f32 (device emits f16)
    fi = top2.astype(np.int64) * C + slot  # [T, 2]
    out = Yf[fi[:, 0]] + Yf[fi[:, 1]]
    return out.reshape(B, S, D).astype(x.dtype), res


def kernel(**inputs) -> np.ndarray:
    out, _ = run(inputs, trace=False)
    return out



# revision 8
# speedup vs baseline: 1.0675x; 1.0014x over previous
"""MoE (8 experts, top-2, D=1024, H=2048, T=8192) on 8 trn2 cores.

Strategy: hidden-dim-split (H-split) expert sharding with host-side routing.
  Unlike pure expert-parallel (one expert per core, padded to the max expert
  load C=max_e ceil128(c_e)), EVERY core processes ALL token-expert pairs but
  only an H/8 = 256 slice of every expert's hidden dimension:
      core c holds w1[e, c*256:(c+1)*256, :], w3[e, ...], w2[e, :, c*256:...]
      for all 8 experts (same 12MB weight footprint as one full expert).
  Per-core work is then EXACTLY Sum_e c_e rows at H/8 regardless of routing
  imbalance (perfect balance, identical program on all cores = SPMD-safe),
  vs max_e ceil128(c_e) rows at full H before.

  Per-expert phase-A blocks use exact token counts (block widths >= ~255 so
  the matmul stream never becomes LDWEIGHTS-bound); phase B works on
  128-aligned subtiles, with the tail padding of hT zeroed via memset.

  Each core emits a PARTIAL y (its H-slice's contribution, cw-scaled, f16);
  the host sums the 8 partials and gathers the top-2 combine.

  Activation flow per block (tokens tb<=512 of expert e, feature-major):
      h1T = w1_e_slice @ xT   [256, tb] (2 psum m-chunks, 8 K-chunks each)
      hT  = silu(h1T) * h3T   [256, tb] bf16
      y  += (hT.T chunks) @ w2_e_sliceT  [tb, 1024] (2 K-chunks), cw-scaled
"""

import sys
import types
from contextlib import ExitStack

import ml_dtypes
import numpy as np

import concourse.bass as bass
import concourse.tile as tile
from concourse import bacc, mybir
from concourse.bass_utils import run_bass_kernel_spmd


def install_axon_hooks_shim():
    """The container's antenv stub lacks axon_hooks, which
    run_bass_kernel_spmd imports whenever tracing is requested (including
    via the BASS_TRACE env var). Recreate it and register the NTFF
    profiling hook if the axon PJRT .so is present."""
    try:
        import antenv
    except ImportError:
        return False
    if "antenv.axon_hooks" in sys.modules:
        return sys.modules["antenv.axon_hooks"]._hook is not None
    mod = types.ModuleType("antenv.axon_hooks")
    mod._hook = None
    mod.set_axon_ntff_profile_hook = lambda h: setattr(mod, "_hook", h)
    mod.get_axon_ntff_profile_hook = lambda: mod._hook
    sys.modules["antenv.axon_hooks"] = mod
    antenv.axon_hooks = mod
    try:
        from trn_agent_boot.trn_boot import _ntff_profile_via_ctypes

        mod.set_axon_ntff_profile_hook(
            _ntff_profile_via_ctypes("/opt/axon/libaxon_pjrt.so")
        )
    except Exception:
        pass
    return mod._hook is not None


install_axon_hooks_shim()

E = 8  # experts == cores
D = 1024
H = 2048
HS = H // E  # per-core hidden slice
TOP_K = 2
KA = D // 128  # contraction chunks for matmul 1
KM = HS // 128  # m-chunks per expert (phase A) == contraction chunks (phase B)

BF16 = mybir.dt.bfloat16
F16 = mybir.dt.float16
F32 = mybir.dt.float32

_CACHE: dict[tuple, object] = {}


def _route(x2d: np.ndarray, router_w: np.ndarray):
    """Float64 router. Returns per-expert token lists, per-expert combine
    weights, and top-2 expert ids per token."""
    T = x2d.shape[0]
    logits = x2d.astype(np.float64) @ router_w.astype(np.float64).T  # [T, E]
    order = np.argsort(-logits, axis=1, kind="stable")
    top2 = order[:, :TOP_K]  # [T, 2]
    lt = np.take_along_axis(logits, top2, axis=1)
    m = lt.max(axis=1, keepdims=True)
    ex = np.exp(lt - m)
    cw = (ex / ex.sum(axis=1, keepdims=True)).astype(np.float32)  # [T, 2]

    rows = []  # rows[e]: token ids routed to expert e (ascending)
    cw_e = []  # cw_e[e]: combine weight per routed token
    slot = np.empty((T, TOP_K), np.int64)  # slot[t, k]: row within expert batch
    for e in range(E):
        r = np.where((top2[:, 0] == e) | (top2[:, 1] == e))[0]
        k = np.where(top2[r, 0] == e, 0, 1)
        rows.append(r)
        cw_e.append(cw[r, k])
        slot[r, k] = np.arange(len(r))
    return rows, cw_e, top2, slot


def _expert_blocks(ce: int):
    """Split an expert's ce tokens into phase-A block widths. All blocks
    start 128-aligned; every block is >= 129 wide (>=255 for sane loads) so
    matmuls stay streaming-bound, not LDWEIGHTS-bound."""
    blocks, off = [], 0
    rem = ce
    while rem >= 640:
        blocks.append((off, 512))
        off += 512
        rem -= 512
    if rem == 0:
        pass
    elif rem <= 512:
        blocks.append((off, rem))
    else:  # 512 < rem < 640: split so neither piece is tiny
        blocks.append((off, 384))
        blocks.append((off + 384, rem - 384))
    return blocks


def _plan(loads: tuple[int, ...]):
    """Global block plan: list of (expert, global_off, width, padded_width),
    per-expert 128-aligned segment offsets, and the padded total."""
    plan = []
    seg_off = []
    off = 0
    for e, ce in enumerate(loads):
        seg_off.append(off)
        se128 = -(-ce // 128) * 128
        blocks = _expert_blocks(ce)
        for i, (boff, w) in enumerate(blocks):
            wp = w if i < len(blocks) - 1 else w + (se128 - ce)
            plan.append((e, off + boff, w, wp))
        off += se128
    return plan, seg_off, off


def _build(loads: tuple[int, ...]):
    """Build + compile the (shared, SPMD) per-core Bass program for the given
    per-expert token loads."""
    plan, seg_off, TT = _plan(loads)
    NB = len(plan)
    nsub = TT // 128

    nc = bacc.Bacc("TRN2", target_bir_lowering=False, debug=False)

    # x, per block, partition-major: [NB][128][KA][512] (only [:w] valid)
    xtp = nc.declare_dram_parameter("xtp", [NB, 128, KA, 512], BF16, isOutput=False)
    # per-expert weight slices (expert 0's w1/w3 split per m-chunk so the
    # first matmul only waits on a 0.25MB transfer)
    w1p0 = [
        nc.declare_dram_parameter(f"w1e0m{m}", [128, KA, 128], BF16, isOutput=False)
        for m in range(KM)
    ]
    w3p0 = [
        nc.declare_dram_parameter(f"w3e0m{m}", [128, KA, 128], BF16, isOutput=False)
        for m in range(KM)
    ]
    w1ps = [
        nc.declare_dram_parameter(f"w1e{e}", [128, KA, HS], BF16, isOutput=False)
        for e in range(1, E)
    ]
    w3ps = [
        nc.declare_dram_parameter(f"w3e{e}", [128, KA, HS], BF16, isOutput=False)
        for e in range(1, E)
    ]
    w2ps = [
        nc.declare_dram_parameter(f"w2e{e}", [128, KM, D], BF16, isOutput=False)
        for e in range(E)
    ]
    cwt = nc.declare_dram_parameter("cwt", [128, nsub], F32, isOutput=False)
    y = nc.declare_dram_parameter("y", [TT, D], F16, isOutput=True)
    y_v = y.rearrange("(n p) d -> n p d", p=128)  # [nsub, 128, D]

    with ExitStack() as ctx:
        tc = ctx.enter_context(tile.TileContext(nc))
        wpool = ctx.enter_context(tc.tile_pool(name="weights", bufs=1))
        xpool = ctx.enter_context(tc.tile_pool(name="x", bufs=7))
        hpool = ctx.enter_context(tc.tile_pool(name="h", bufs=2))
        spool = ctx.enter_context(tc.tile_pool(name="s", bufs=3))
        ypool = ctx.enter_context(tc.tile_pool(name="y", bufs=8))
        ppool = ctx.enter_context(tc.tile_pool(name="psum", bufs=2, space="PSUM"))

        # first block's activations first — they gate the first matmul
        def xts_load(bi, w):
            xa = xpool.tile([128, KA, w], BF16, tag="xts")
            nc.sync.dma_start(xa[:], xtp[bi, :, :, 0:w])
            return xa

        # DMA issue order is load-bearing: descriptors are processed in ring
        # order, so the 12MB weight burst must be interleaved with prefetches
        # of the first few x blocks or block 1..5's activations arrive ~40µs
        # in and the PE stalls (~25µs measured). Weights for expert e are only
        # needed ~(40*e)µs into the kernel; x blocks every ~10µs.
        xts_pre = {0: xts_load(0, plan[0][2])}

        def xpre(bi):
            if bi < NB and bi not in xts_pre:
                xts_pre[bi] = xts_load(bi, plan[bi][2])

        # expert 0 w1/w3 (split per m-chunk; first matmul waits only on m0)
        w1e0, w3e0 = [], []
        for m in range(KM):
            t1 = wpool.tile([128, KA, 128], BF16, tag=f"w1e0m{m}")
            nc.sync.dma_start(t1[:], w1p0[m][:])
            t3 = wpool.tile([128, KA, 128], BF16, tag=f"w3e0m{m}")
            nc.sync.dma_start(t3[:], w3p0[m][:])
            w1e0.append(t1)
            w3e0.append(t3)
        xpre(1)
        w2t = [wpool.tile([128, KM, D], BF16, tag="w2e0", name="w2e0")]
        nc.sync.dma_start(w2t[0][:], w2ps[0][:])
        cws = wpool.tile([128, nsub], F32, tag="cws")
        nc.sync.dma_start(cws[:], cwt[:])
        xpre(2)

        w1t, w3t = [None], [None]  # e>=1 full-slice tiles
        for e in range(1, E):
            w1t.append(
                wpool.tile([128, KA, HS], BF16, tag=f"w1e{e}", name=f"w1e{e}")
            )
            w3t.append(
                wpool.tile([128, KA, HS], BF16, tag=f"w3e{e}", name=f"w3e{e}")
            )
            w2t.append(
                wpool.tile([128, KM, D], BF16, tag=f"w2e{e}", name=f"w2e{e}")
            )
        issued = set()

        def issue_weights_upto(elim):
            for e in range(1, min(elim, E - 1) + 1):
                if e in issued or loads[e] == 0:
                    continue
                issued.add(e)
                nc.sync.dma_start(w1t[e][:], w1ps[e - 1][:])
                nc.sync.dma_start(w3t[e][:], w3ps[e - 1][:])
                nc.sync.dma_start(w2t[e][:], w2ps[e][:])

        # Only expert 1 pre-issued; experts 2+ are issued just-in-time from
        # inside the block loop (see below) so the weight burst doesn't crowd
        # the early x-in/y-out DMA window (12MB early = PE stalls on ypool).
        issue_weights_upto(1)
        xpre(3)
        xpre(4)

        # PE prewarm: ~16 dummy matmuls on a memset tile during the ~13µs
        # DMA-bound startup so the HAM clock-gate reaches 8/8 before the
        # first real matmul (saves the ~4µs cold-ramp; see tensor-engine
        # docs on the 3.4µs activity window).
        warm_src = spool.tile([128, 512], BF16, tag="warm_src", name="warm_src")
        nc.vector.memset(warm_src[:], 0.0)
        warm_ps = ppool.tile([128, 512], F32, tag="ph1", name="warm_ps")
        NWARM = 20  # ~8 cold + 12 warm MMs ≈ 6µs: ends ~13.3µs, right when
        # the first real matmul's data lands (>3.4µs early would re-throttle
        # the PE; later would push the real work out)
        for i in range(NWARM):
            nc.tensor.matmul(
                warm_ps[:],
                warm_src[:, 0:128],
                warm_src[:, 0:512],
                start=(i == 0),
                stop=(i == NWARM - 1),
            )

        def w13_slice(e, m, a):
            if e == 0:
                return w1e0[m][:, a, :], w3e0[m][:, a, :]
            return (
                w1t[e][:, a, bass.ts(m, 128)],
                w3t[e][:, a, bass.ts(m, 128)],
            )

        cur_e = 0
        for bi, (e, goff, w, wp) in enumerate(plan):
            # Just-in-time weight issuance, spread across the sync stream so
            # descriptors land ~2 experts (~80µs) before they're needed
            # without crowding the early x/y ring traffic: expert e+2's
            # weights go out at expert e's SECOND block, e2 extra-early.
            if bi == 1:
                issue_weights_upto(2)
            if e != cur_e:
                cur_e = e
                issue_weights_upto(e + 2)
            elif bi > 0 and plan[bi - 1][0] == e and goff == plan[bi - 1][1] + 512:
                pass  # (position marker: nothing to issue mid-expert)
            xts = xts_pre.pop(bi) if bi in xts_pre else xts_load(bi, plan[bi][2])

            hts = hpool.tile([128, KM, wp], BF16, tag="hts")
            if wp > w:  # zero hT padding so phase B reads zeros, not garbage
                nc.vector.memset(hts[:, :, w:wp], 0.0)

            # phase A: h1T/h3T chunks + silu * mul -> hts
            for m in range(KM):
                ph1 = ppool.tile([128, w], F32, tag="ph1")
                ph3 = ppool.tile([128, w], F32, tag="ph3")
                for a in range(KA):
                    w1s, _ = w13_slice(e, m, a)
                    nc.tensor.matmul(
                        ph1[:], w1s, xts[:, a, :], start=(a == 0), stop=(a == KA - 1)
                    )
                for a in range(KA):
                    _, w3s = w13_slice(e, m, a)
                    nc.tensor.matmul(
                        ph3[:], w3s, xts[:, a, :], start=(a == 0), stop=(a == KA - 1)
                    )
                sil = spool.tile([128, w], BF16, tag="sil")
                nc.scalar.activation(
                    sil[:], ph1[:], mybir.ActivationFunctionType.Silu
                )
                nc.vector.tensor_mul(hts[:, m, 0:w], sil[:], ph3[:])

            # phase B: y += (hT.T) @ w2T for this block's subtiles, cw-scaled
            for n in range(wp // 128):
                nsl = bass.ts(n, 128)
                gn = goff // 128 + n
                py0 = ppool.tile([128, 512], F32, tag="py0")
                py1 = ppool.tile([128, 512], F32, tag="py1")
                for m in range(KM):
                    nc.tensor.matmul(
                        py0[:],
                        hts[:, m, nsl],
                        w2t[e][:, m, 0:512],
                        start=(m == 0),
                        stop=(m == KM - 1),
                    )
                    nc.tensor.matmul(
                        py1[:],
                        hts[:, m, nsl],
                        w2t[e][:, m, 512:1024],
                        start=(m == 0),
                        stop=(m == KM - 1),
                    )
                # drain the two psum halves on DIFFERENT engines (DVE +
                # Scalar) so phase B stays tensor-paced: one DVE op per
                # subtile (658ns) < 4 matmuls (853ns), whereas two serialized
                # DVE drains (1.3µs) starve the PE and let HAM re-throttle.
                # y DMAs stay on the sync engine's rings; weights go out on
                # GpSimd's queues so y descriptors never queue behind the
                # 12MB weight burst (which stalls ypool/psum -> 2-3µs PE
                # gaps per early expert).
                ys0 = ypool.tile([128, 512], F16, tag="ys0")
                nc.vector.tensor_scalar_mul(ys0[:], py0[:], cws[:, gn : gn + 1])
                nc.sync.dma_start(y_v[gn][:, 0:512], ys0[:])
                ys1 = ypool.tile([128, 512], F16, tag="ys1")
                nc.scalar.activation(
                    ys1[:],
                    py1[:],
                    mybir.ActivationFunctionType.Copy,
                    scale=cws[:, gn : gn + 1],
                )
                nc.sync.dma_start(y_v[gn][:, 512:1024], ys1[:])

    nc.compile()
    return nc


def _get(loads: tuple[int, ...]):
    if loads not in _CACHE:
        _CACHE[loads] = _build(loads)
    return _CACHE[loads]


def _prepare_core_inputs(x2d, w1, w2, w3, rows, cw_e):
    """Build per-core input maps. xtp/cwt are identical on every core; the
    weight slices differ (core c gets hidden rows [c*HS, (c+1)*HS) of every
    expert)."""
    bf = ml_dtypes.bfloat16
    loads = tuple(len(r) for r in rows)
    plan, seg_off, TT = _plan(loads)
    NB = len(plan)
    nsub = TT // 128

    xtp = np.zeros((NB, 128, KA, 512), bf)
    cwt = np.zeros((TT,), np.float32)
    for e in range(E):
        ce = loads[e]
        if ce == 0:
            continue
        xe = x2d[rows[e]].T.astype(bf)  # [D, ce]
        xpm = xe.reshape(KA, 128, ce).transpose(1, 0, 2)  # [128, KA, ce]
        cwt[seg_off[e] : seg_off[e] + ce] = cw_e[e]
        for bi, (be, goff, w, wp) in enumerate(plan):
            if be != e:
                continue
            boff = goff - seg_off[e]
            xtp[bi, :, :, 0:w] = xpm[:, :, boff : boff + w]
    cwt2 = np.ascontiguousarray(cwt.reshape(nsub, 128).T)

    w1b = w1.astype(bf)  # [E, H, D]
    w3b = w3.astype(bf)
    w2b = w2.astype(bf)  # [E, D, H]

    in_maps = []
    for c in range(E):
        hs = slice(c * HS, (c + 1) * HS)
        m = {"xtp": xtp, "cwt": cwt2}
        for e in range(E):
            # w1/w3 slice: [D, HS] -> partition-major [128, KA, HS]
            w1pm = np.ascontiguousarray(
                w1b[e, hs, :].T.reshape(KA, 128, HS).transpose(1, 0, 2)
            )
            w3pm = np.ascontiguousarray(
                w3b[e, hs, :].T.reshape(KA, 128, HS).transpose(1, 0, 2)
            )
            if e == 0:
                for mm in range(KM):
                    m[f"w1e0m{mm}"] = np.ascontiguousarray(
                        w1pm[:, :, mm * 128 : (mm + 1) * 128]
                    )
                    m[f"w3e0m{mm}"] = np.ascontiguousarray(
                        w3pm[:, :, mm * 128 : (mm + 1) * 128]
                    )
            else:
                m[f"w1e{e}"] = w1pm
                m[f"w3e{e}"] = w3pm
            # w2 slice: [HS, D] -> [128, KM, D]
            m[f"w2e{e}"] = np.ascontiguousarray(
                w2b[e, :, hs].T.reshape(KM, 128, D).transpose(1, 0, 2)
            )
        in_maps.append(m)
    return in_maps, seg_off, TT


def run(inputs: dict, trace: bool = False, trace_cores=None):
    """Core implementation; returns (output, BassKernelResults)."""
    x = np.asarray(inputs["x"])
    router_w = np.asarray(inputs["router_w"], np.float32)
    w1 = np.asarray(inputs["w1"], np.float32)
    w2 = np.asarray(inputs["w2"], np.float32)
    w3 = np.asarray(inputs["w3"], np.float32)

    B, S, _ = x.shape
    assert x.shape[-1] == D and router_w.shape == (E, D), (x.shape, router_w.shape)
    assert w1.shape == (E, H, D) and w3.shape == (E, H, D) and w2.shape == (E, D, H)
    x2d = np.ascontiguousarray(x.reshape(-1, D).astype(np.float32))

    rows, cw_e, top2, slot = _route(x2d, router_w)
    loads = tuple(len(r) for r in rows)

    nc = _get(loads)
    in_maps, seg_off, TT = _prepare_core_inputs(x2d, w1, w2, w3, rows, cw_e)
    res = run_bass_kernel_spmd(
        nc,
        in_maps,
        list(range(E)),
        trace=trace,
        trace_cores=trace_cores,
    )

    # sum the 8 per-core H-slice partials, then gather the top-2 combine
    Ysum = np.zeros((TT, D), np.float32)
    for c in range(E):
        Ysum += np.asarray(res.results[c]["y"], np.float32)
    off = np.asarray(seg_off, np.int64)
    fi = off[top2] + slot  # [T, 2] global row ids
    out = Ysum[fi[:, 0]] + Ysum[fi[:, 1]]
    return out.reshape(B, S, D).astype(x.dtype), res


def kernel(**inputs) -> np.ndarray:
    out, _ = run(inputs, trace=False)
    return out
